# revision 2
# baseline (speedup 1.0000x reference)
"""TRN2 Bass kernel for nn_CrispComposition: out[b,o] = max_i min(m[b,i], w[i,o]).

Full-input contract: kernel(m, weight) takes the full [2048, 512] m and
[512, 256] weight, shards m row-wise across 8 NeuronCores (data-parallel,
weight replicated), runs a Bass kernel per core via run_bass_kernel_spmd,
and concatenates the per-core outputs into the full [2048, 256] result.

Per-core algorithm (fp32-exact):
  - m shard [256, 512] lives as two [128, 512] SBUF tiles (partition = batch).
  - acc[bt] [128, 256] accumulates over the contraction index i:
      i == 0 : acc = min(wb_0, m[:, 0])            (tensor_scalar)
      i  > 0 : acc = max(min(wb_i, m[:, i]), acc)  (scalar_tensor_tensor)
    where wb_i [128, 256] is weight row i DMA-broadcast across the 128
    partitions and m[:, i] is a [128, 1] per-partition scalar operand.
All arithmetic is fp32 min/max, so the result is bit-exact vs the fp32
reference.

This file also carries two compatibility patches for the container's
walrus build (it rejects EVENT_SEMAPHORE_RANGE_CLEAR and any instruction
with more than one attached sem-wait); see _apply_walrus_patches /
_split_excess_waits.
"""

import sys
from contextlib import ExitStack

for _p in ("/opt/trn_rl_repo", "/root/.axon_site/_ro/trn_rl_repo"):
    if _p not in sys.path:
        sys.path.insert(0, _p)

import numpy as np

import concourse.bass as bass
import concourse.mybir as mybir
import concourse.tile as tile
from concourse import bass_utils

N_CORES = 8
P = 128
BATCH = 2048
I_DIM = 512
O_DIM = 256
B_CORE = BATCH // N_CORES  # 256 rows per core

# ---------------------------------------------------------------------------
# walrus compatibility
# ---------------------------------------------------------------------------

_PATCHED = False
_split_counter = [0]


def _apply_walrus_patches():
    """The bundled walrus_driver rejects EVENT_SEMAPHORE_RANGE_CLEAR
    ("ISA wrong length").  It is only emitted for semaphore recycling at
    scope exit; nothing executes afterwards in a one-shot kernel, so skip
    the device-side clear and keep the Python-side bookkeeping."""
    global _PATCHED
    if _PATCHED:
        return
    _PATCHED = True

    def _clear_and_free_semaphores(self, sems):
        if not sems:
            return
        sem_nums = [s.num if hasattr(s, "num") else s for s in sems]
        self._state.prepend_free_semaphores(sem_nums)
        for poison_set in self._tile_sem_poison_stack:
            poison_set.update(sem_nums)

    bass.Bass.clear_and_free_semaphores = _clear_and_free_semaphores


def _split_excess_waits(nc, limit=1):
    """The bundled walrus_driver accepts at most one sem-wait per
    instruction ("Too many sync wait commands").  Move excess waits onto
    wait-only Drain instructions inserted just before, on the same engine
    (program order on the engine makes this semantically identical)."""
    n_split = 0
    for fn in nc.m.functions:
        for bb in fn.blocks:
            new_insts = []
            for inst in bb.instructions:
                si = inst.sync_info
                waits = list(si.on_wait) if si is not None and si.on_wait else []
                if len(waits) > limit:
                    extras, keep = waits[:-limit], waits[-limit:]
                    for w in extras:
                        _split_counter[0] += 1
                        d = mybir.InstDrain(
                            name=f"I-waitsplit-{_split_counter[0]}",
                            opcode="Drain",
                            engine=inst.engine,
                            debug=inst.debug,
                            ins=[],
                            outs=[],
                            sync_info=mybir.SyncInfo(on_wait=[w], on_update=[]),
                        )
                        new_insts.append(d)
                        n_split += 1
                    inst.sync_info = mybir.SyncInfo(
                        on_wait=keep, on_update=list(si.on_update or [])
                    )
                new_insts.append(inst)
            bb.instructions = new_insts
    return n_split


# ---------------------------------------------------------------------------
# kernel
# ---------------------------------------------------------------------------


def _build_crisp_kernel(tc, out_ap, m_ap, w_ap, wb_bufs=12):
    nc = tc.nc
    nbt = B_CORE // P

    with ExitStack() as ctx:
        const_pool = ctx.enter_context(tc.tile_pool(name="const", bufs=1))
        wb_pool = ctx.enter_context(tc.tile_pool(name="wb", bufs=wb_bufs))

        m_tiles = []
        for bt in range(nbt):
            mt = const_pool.tile(
                [P, I_DIM], mybir.dt.float32, name=f"mt{bt}", tag=f"m{bt}"
            )
            nc.sync.dma_start(out=mt, in_=m_ap[bt * P : (bt + 1) * P, :])
            m_tiles.append(mt)

        acc_tiles = [
            const_pool.tile(
                [P, O_DIM], mybir.dt.float32, name=f"acc{bt}", tag=f"acc{bt}"
            )
            for bt in range(nbt)
        ]

        for i in range(I_DIM):
            wb = wb_pool.tile([P, O_DIM], mybir.dt.float32, name=f"wb{i}", tag="wb")
            nc.sync.dma_start(
                out=wb, in_=w_ap[i : i + 1, :].broadcast_to([P, O_DIM])
            )
            for bt in range(nbt):
                if i == 0:
                    nc.vector.tensor_scalar(
                        out=acc_tiles[bt][:, :],
                        in0=wb[:, :],
                        scalar1=m_tiles[bt][:, i : i + 1],
                        scalar2=None,
                        op0=mybir.AluOpType.min,
                    )
                else:
                    nc.vector.scalar_tensor_tensor(
                        out=acc_tiles[bt][:, :],
                        in0=wb[:, :],
                        scalar=m_tiles[bt][:, i : i + 1],
                        in1=acc_tiles[bt][:, :],
                        op0=mybir.AluOpType.min,
                        op1=mybir.AluOpType.max,
                    )

        for bt in range(nbt):
            nc.sync.dma_start(
                out=out_ap[bt * P : (bt + 1) * P, :], in_=acc_tiles[bt]
            )


def _build_nc():
    _apply_walrus_patches()
    nc = bass.Bass("TRN2", target_bir_lowering=False, debug=False)
    m_t = nc.dram_tensor("m_shard", [B_CORE, I_DIM], mybir.dt.float32,
                         kind="ExternalInput")
    w_t = nc.dram_tensor("w", [I_DIM, O_DIM], mybir.dt.float32,
                         kind="ExternalInput")
    out_t = nc.dram_tensor("out_shard", [B_CORE, O_DIM], mybir.dt.float32,
                           kind="ExternalOutput")
    with tile.TileContext(nc) as tc:
        _build_crisp_kernel(tc, out_t.ap(), m_t.ap(), w_t.ap())
    _split_excess_waits(nc)
    return nc


_CACHED = {}


def _run(m, weight, trace=False, **kwargs):
    m = np.ascontiguousarray(m, dtype=np.float32)
    w = np.ascontiguousarray(weight, dtype=np.float32)

    if "nc" not in _CACHED:
        _CACHED["nc"] = _build_nc()
    nc = _CACHED["nc"]

    in_maps = [
        {"m_shard": m[c * B_CORE : (c + 1) * B_CORE, :], "w": w}
        for c in range(N_CORES)
    ]
    res = bass_utils.run_bass_kernel_spmd(
        nc, in_maps, core_ids=list(range(N_CORES)), trace=trace, **kwargs
    )
    out = np.concatenate(
        [res.results[c]["out_shard"] for c in range(N_CORES)], axis=0
    )
    return out, res


def kernel(m, weight):
    out, _ = _run(m, weight, trace=False)
    return out


# revision 4
# speedup vs baseline: 1.3234x; 1.3234x over previous
"""TRN2 Bass kernel for nn_CrispComposition: out[b,o] = max_i min(m[b,i], w[i,o]).

Full-input contract: kernel(m, weight) takes the full [2048, 512] m and
[512, 256] weight, shards m row-wise across 8 NeuronCores (data-parallel,
weight replicated), runs a Bass kernel per core via run_bass_kernel_spmd,
and concatenates the per-core outputs into the full [2048, 256] result.

Per-core algorithm (fp32-exact):
  - m shard [256, 512] lives as two [128, 512] SBUF tiles (partition = batch).
  - acc[bt] [128, 256] accumulates over the contraction index i:
      i == 0 : acc = min(wb_0, m[:, 0])            (tensor_scalar)
      i  > 0 : acc = max(min(wb_i, m[:, i]), acc)  (scalar_tensor_tensor)
    where wb_i [128, 256] is weight row i DMA-broadcast across the 128
    partitions and m[:, i] is a [128, 1] per-partition scalar operand.
All arithmetic is fp32 min/max, so the result is bit-exact vs the fp32
reference.

This file also carries two compatibility patches for the container's
walrus build (it rejects EVENT_SEMAPHORE_RANGE_CLEAR and any instruction
with more than one attached sem-wait); see _apply_walrus_patches /
_split_excess_waits.
"""

import sys
from contextlib import ExitStack

for _p in ("/opt/trn_rl_repo", "/root/.axon_site/_ro/trn_rl_repo"):
    if _p not in sys.path:
        sys.path.insert(0, _p)

import numpy as np

import concourse.bass as bass
import concourse.mybir as mybir
import concourse.tile as tile
from concourse import bass_utils

N_CORES = 8
P = 128
BATCH = 2048
I_DIM = 512
O_DIM = 256
B_CORE = BATCH // N_CORES  # 256 rows per core

# ---------------------------------------------------------------------------
# walrus compatibility
# ---------------------------------------------------------------------------

_PATCHED = False
_split_counter = [0]


def _apply_walrus_patches():
    """The bundled walrus_driver rejects EVENT_SEMAPHORE_RANGE_CLEAR
    ("ISA wrong length").  It is only emitted for semaphore recycling at
    scope exit; nothing executes afterwards in a one-shot kernel, so skip
    the device-side clear and keep the Python-side bookkeeping."""
    global _PATCHED
    if _PATCHED:
        return
    _PATCHED = True

    def _clear_and_free_semaphores(self, sems):
        if not sems:
            return
        sem_nums = [s.num if hasattr(s, "num") else s for s in sems]
        self._state.prepend_free_semaphores(sem_nums)
        for poison_set in self._tile_sem_poison_stack:
            poison_set.update(sem_nums)

    bass.Bass.clear_and_free_semaphores = _clear_and_free_semaphores


_ENGINE_PROC_NAME = {
    "EngineType.Pool": "Pool",
    "EngineType.Activation": "Activation",
    "EngineType.PE": "PE",
    "EngineType.DVE": "DVE",
    "EngineType.SP": "SP",
}

# Engines whose instructions execute strictly one-at-a-time (the DVE pipe
# drains between ops; ACT likewise), so a wait on the engine's *own* proc
# semaphore is implied by program order.
_SERIAL_ENGINES = {"DVE", "Activation"}


def _wait_proc(w):
    name = w.ant_name or ""
    return name.rsplit("_", 1)[0]


def _prune_redundant_waits(nc):
    """Tile's wait assignment is per-proc minimal but not transitively
    minimal.  Two classes of waits are provably redundant here and are
    dropped so the one-wait-per-instruction walrus limit is met without
    extra carrier drains:
      - a compute op on a serial engine (DVE/ACT) waiting on its own
        engine's proc semaphore: program order already guarantees it;
      - a DMACopy that waits on both a DVE proc sem (its buffer's consumers)
        and a DMAHW proc sem (the previous DMA that wrote the slot): the
        consumers only ran after that DMA completed, so the DVE wait
        transitively covers the DMAHW wait."""
    for fn in nc.m.functions:
        for bb in fn.blocks:
            for inst in bb.instructions:
                si = inst.sync_info
                if si is None or not si.on_wait or len(si.on_wait) < 2:
                    continue
                waits = list(si.on_wait)
                eng_proc = _ENGINE_PROC_NAME.get(str(inst.engine))
                if eng_proc in _SERIAL_ENGINES:
                    kept = [w for w in waits if _wait_proc(w) != eng_proc]
                    if not kept:  # keep at least one (cheap, satisfied)
                        kept = waits[-1:]
                    waits = kept
                if inst.opcode == "DMACopy" and any(
                    _wait_proc(w) == "DVE" for w in waits
                ):
                    kept = [w for w in waits if not _wait_proc(w).startswith("DMAHW")]
                    if kept:
                        waits = kept
                if len(waits) != len(si.on_wait):
                    inst.sync_info = mybir.SyncInfo(
                        on_wait=waits, on_update=list(si.on_update or [])
                    )


def _split_excess_waits(nc, limit=1):
    """The bundled walrus_driver accepts at most one sem-wait per
    instruction ("Too many sync wait commands").  Move excess waits onto
    wait-only Drain instructions inserted just before, on the same engine
    (program order on the engine makes this semantically identical)."""
    _prune_redundant_waits(nc)
    n_split = 0
    for fn in nc.m.functions:
        for bb in fn.blocks:
            new_insts = []
            for inst in bb.instructions:
                si = inst.sync_info
                waits = list(si.on_wait) if si is not None and si.on_wait else []
                if len(waits) > limit:
                    extras, keep = waits[:-limit], waits[-limit:]
                    for w in extras:
                        _split_counter[0] += 1
                        d = mybir.InstDrain(
                            name=f"I-waitsplit-{_split_counter[0]}",
                            opcode="Drain",
                            engine=inst.engine,
                            debug=inst.debug,
                            ins=[],
                            outs=[],
                            sync_info=mybir.SyncInfo(on_wait=[w], on_update=[]),
                        )
                        new_insts.append(d)
                        n_split += 1
                    inst.sync_info = mybir.SyncInfo(
                        on_wait=keep, on_update=list(si.on_update or [])
                    )
                new_insts.append(inst)
            bb.instructions = new_insts
    return n_split


# ---------------------------------------------------------------------------
# kernel
# ---------------------------------------------------------------------------


def _build_crisp_kernel(tc, out_ap, m_ap, w_ap, wb_bufs=16):
    nc = tc.nc
    nbt = B_CORE // P

    with ExitStack() as ctx:
        const_pool = ctx.enter_context(tc.tile_pool(name="const", bufs=1))
        wb_pool = ctx.enter_context(tc.tile_pool(name="wb", bufs=wb_bufs))

        m_tiles = []
        for bt in range(nbt):
            mt = const_pool.tile(
                [P, I_DIM], mybir.dt.float32, name=f"mt{bt}", tag=f"m{bt}"
            )
            nc.sync.dma_start(out=mt, in_=m_ap[bt * P : (bt + 1) * P, :])
            m_tiles.append(mt)

        acc_tiles = [
            const_pool.tile(
                [P, O_DIM], mybir.dt.float32, name=f"acc{bt}", tag=f"acc{bt}"
            )
            for bt in range(nbt)
        ]

        for i in range(I_DIM):
            wb = wb_pool.tile([P, O_DIM], mybir.dt.float32, name=f"wb{i}", tag="wb")
            nc.sync.dma_start(
                out=wb, in_=w_ap[i : i + 1, :].broadcast_to([P, O_DIM])
            )
            for bt in range(nbt):
                if i == 0:
                    nc.vector.tensor_scalar(
                        out=acc_tiles[bt][:, :],
                        in0=wb[:, :],
                        scalar1=m_tiles[bt][:, i : i + 1],
                        scalar2=None,
                        op0=mybir.AluOpType.min,
                    )
                else:
                    nc.vector.scalar_tensor_tensor(
                        out=acc_tiles[bt][:, :],
                        in0=wb[:, :],
                        scalar=m_tiles[bt][:, i : i + 1],
                        in1=acc_tiles[bt][:, :],
                        op0=mybir.AluOpType.min,
                        op1=mybir.AluOpType.max,
                    )

        for bt in range(nbt):
            nc.sync.dma_start(
                out=out_ap[bt * P : (bt + 1) * P, :], in_=acc_tiles[bt]
            )


def _build_nc():
    _apply_walrus_patches()
    nc = bass.Bass("TRN2", target_bir_lowering=False, debug=False)
    m_t = nc.dram_tensor("m_shard", [B_CORE, I_DIM], mybir.dt.float32,
                         kind="ExternalInput")
    w_t = nc.dram_tensor("w", [I_DIM, O_DIM], mybir.dt.float32,
                         kind="ExternalInput")
    out_t = nc.dram_tensor("out_shard", [B_CORE, O_DIM], mybir.dt.float32,
                           kind="ExternalOutput")
    with tile.TileContext(nc) as tc:
        _build_crisp_kernel(tc, out_t.ap(), m_t.ap(), w_t.ap())
    _split_excess_waits(nc)
    return nc


_CACHED = {}


def _run(m, weight, trace=False, **kwargs):
    m = np.ascontiguousarray(m, dtype=np.float32)
    w = np.ascontiguousarray(weight, dtype=np.float32)

    if "nc" not in _CACHED:
        _CACHED["nc"] = _build_nc()
    nc = _CACHED["nc"]

    in_maps = [
        {"m_shard": m[c * B_CORE : (c + 1) * B_CORE, :], "w": w}
        for c in range(N_CORES)
    ]
    res = bass_utils.run_bass_kernel_spmd(
        nc, in_maps, core_ids=list(range(N_CORES)), trace=trace, **kwargs
    )
    out = np.concatenate(
        [res.results[c]["out_shard"] for c in range(N_CORES)], axis=0
    )
    return out, res


def kernel(m, weight):
    out, _ = _run(m, weight, trace=False)
    return out
